# revision 17
# baseline (speedup 1.0000x reference)
"""Multi-head attention (B=8, N=1024, DIM=768, H=12) on 8 Trainium2 NeuronCores.

Sharding: pure data-parallel over the batch dimension — core c computes batch
element c end-to-end (qkv projection, softmax attention, output projection).
No collectives needed.

Numerics: matmul inputs in bf16 (x, weights, q/k, v, exp(P)) with fp32 PSUM
accumulation; softmax denominator, reciprocal, normalization and bias in
fp32. bf16 weight loads are FWL-accelerated and overlap matmuls via the PE
background weight buffer (fp32/f32r self-loading matmuls serialize a 213ns
LDWEIGHTS with every matmul - measured; that was the v2 bottleneck).

Schedule: stage-1 projection matmuls are interleaved INTO the attention loop
as PE filler so the tensor engine never idles while ScalarE computes exp (an
idle PE re-throttles to 1.2 GHz via HAM and doubles all matmul times).

  per head-pair t (heads 2t, 2t+1):
      qk pair-tile:  qkT[:, 2t], qkT[:, 2t+1] = [Wq_t; Wk_t] @ x^T
      v pair-slice:  v[:, :, 2t:2t+2] = x @ Wv_t^T   (+ ones column)
      per head: S^T[j,i] = k^T q (j on partitions), P = exp(S*scale) in one
      ACTIVATE per j-tile; PV accumulates OT_aug [d+1, i] with row 64 =
      softmax denominator l (ones column of v_aug).
      r-chain: l row -> SBUF (fp32) -> DRAM bounce -> broadcast DMA to all
      128 partitions -> reciprocal_approx_fast -> normalize fused into the
      PSUM->SBUF copy of OT (bf16 out).
  stage 3: y = OT^T @ WpT + bias (fp32)
"""

import os
import sys

for _p in ("/opt/trn_rl_repo",):
    if _p not in sys.path:
        sys.path.insert(0, _p)

import ml_dtypes
import numpy as np

import concourse.bass as bass
import concourse.tile as tile
from concourse import bacc, mybir

B, N, DIM, H = 8, 1024, 768, 12
D = DIM // H  # 64
SCALE = D ** -0.5
P = 128
KT = DIM // P        # 6 contraction tiles over dim
NT = N // P          # 8 tiles over sequence
NPAIR = H // 2       # 6 head pairs
FP = mybir.dt.float32
BF = mybir.dt.bfloat16
MMDT = BF
NP_MMDT = ml_dtypes.bfloat16


def _chunks(total, size):
    return [(lo, min(lo + size, total)) for lo in range(0, total, size)]


def build_nc():
    nc = bacc.Bacc(None, target_bir_lowering=False)
    xT = nc.dram_tensor("xT", [DIM, N], MMDT, kind="ExternalInput")
    # wqkT columns are pair-blocked: [q_t | k_t] of 128 cols each, t=0..5
    wqkT = nc.dram_tensor("wqkT", [DIM, 2 * DIM], MMDT, kind="ExternalInput")
    wvT = nc.dram_tensor("wvT", [DIM, DIM], MMDT, kind="ExternalInput")
    wpT = nc.dram_tensor("wpT", [DIM, DIM], MMDT, kind="ExternalInput")
    bias = nc.dram_tensor("bias", [1, DIM], FP, kind="ExternalInput")
    y = nc.dram_tensor("y", [N, DIM], FP, kind="ExternalOutput")

    with tile.TileContext(nc) as tc:
        with nc.allow_low_precision(reason="bf16 matmul inputs"):
            _body(tc, xT, wqkT, wvT, wpT, bias, y)
    nc.compile()
    return nc


def _body(tc, xT, wqkT, wvT, wpT, bias, y):
    nc = tc.nc
    Exp = mybir.ActivationFunctionType.Exp
    Mult = mybir.AluOpType.mult
    Add = mybir.AluOpType.add

    from contextlib import ExitStack
    with tc.tile_pool(name="persist", bufs=1) as persist:
      with ExitStack() as s12:
        s1w = s12.enter_context(tc.tile_pool(name="s1w", bufs=1))
        expp = s12.enter_context(tc.tile_pool(name="expp", bufs=3))
        rp = s12.enter_context(tc.tile_pool(name="rp", bufs=2))
        s1ps = s12.enter_context(tc.tile_pool(name="s1ps", bufs=1, space="PSUM"))
        stps = s12.enter_context(tc.tile_pool(name="stps", bufs=2, space="PSUM"))
        otps = s12.enter_context(tc.tile_pool(name="otps", bufs=3, space="PSUM"))

        # qkT_sb tile index 2t = q of pair t, 2t+1 = k of pair t; rows (h%2,d)
        qkT_sb = persist.tile([P, 2 * KT, N], MMDT)     # 24 KB/part
        v_sb = persist.tile([P, NT, H, D + 1], MMDT)    # 12.7 KB/part
        oT_sb = persist.tile([P, KT, N], MMDT)          # 12 KB/part
        bias_sb = persist.tile([P, DIM], FP)            # 3 KB/part
        ones_f32r = persist.tile([1, P], mybir.dt.float32r)
        ones_stg = persist.tile([1, P], FP)
        nc.sync.dma_start(out=bias_sb, in_=bias[:].to_broadcast((P, DIM)))
        nc.vector.memset(v_sb[:, :, :, D], 1.0)
        nc.vector.memset(ones_stg, 1.0)
        nc.vector.tensor_copy(out=ones_f32r, in_=ones_stg)

        xT_sb = s1w.tile([P, KT, N], MMDT)              # 12 KB/part
        wqkT_sb = s1w.tile([P, KT, 2 * DIM], MMDT)      # 18 KB/part
        wvT_sb = s1w.tile([P, KT, DIM], MMDT)           # 9 KB/part

        xTr = xT[:].rearrange("(t p) n -> t p n", p=P)
        wqkr = wqkT[:].rearrange("(t p) m -> t p m", p=P)
        wvr = wvT[:].rearrange("(t p) m -> t p m", p=P)

        # DMA order: x first (everything needs it), wv (for the v prologue),
        # then per-pair wqk slices so pair-0 unblocks earliest. Batched into
        # few large descriptors: serial DMA_DIRECT2D triggers (~0.6us each)
        # were the v4 lead-in bottleneck.
        nc.sync.dma_start(out=xT_sb, in_=xTr.rearrange("t p n -> p t n"))
        nc.sync.dma_start(
            out=wqkT_sb[:, :, 0:256],
            in_=wqkr.rearrange("t p m -> p t m")[:, :, 0:256],
        )
        nc.sync.dma_start(out=wvT_sb, in_=wvr.rearrange("t p m -> p t m"))
        for t in range(1, NPAIR):
            nc.sync.dma_start(
                out=wqkT_sb[:, :, t * 256:(t + 1) * 256],
                in_=wqkr.rearrange("t p m -> p t m")[:, :, t * 256:(t + 1) * 256],
            )

        # ---- PE work generators (filler units of ~0.5-1.3us of matmuls) ----
        def gen_qk(t):
            """qk pair-tile t -> qkT_sb[:, 2t] (q) and [:, 2t+1] (k)."""
            for which in range(2):
                for lo, hi in _chunks(N, 512):
                    ps = s1ps.tile([P, 512], FP, tag="s1")
                    for k in range(KT):
                        nc.tensor.matmul(
                            ps,
                            wqkT_sb[:, k, t * 256 + which * P:
                                    t * 256 + (which + 1) * P],
                            xT_sb[:, k, lo:hi],
                            start=(k == 0),
                            stop=(k == KT - 1),
                        )
                    nc.vector.tensor_copy(
                        out=qkT_sb[:, 2 * t + which, lo:hi], in_=ps)
                    yield

        def gen_v_all():
            """v for ALL heads -> v_sb[:, :, :, 0:D]; full-width moving
            chunks (N=512/256) so the PE runs at peak issue rate."""
            for j in range(NT):
                for lo, hi in _chunks(DIM, 512):
                    ps = s1ps.tile([P, 512], FP, tag="s1")
                    for k in range(KT):
                        nc.tensor.matmul(
                            ps[:, 0:hi - lo],
                            xT_sb[:, k, j * P:(j + 1) * P],
                            wvT_sb[:, k, lo:hi],
                            start=(k == 0),
                            stop=(k == KT - 1),
                        )
                    nc.vector.tensor_copy(
                        out=v_sb[:, j, lo // D:hi // D, 0:D],
                        in_=ps[:, 0:hi - lo].rearrange(
                            "p (h d) -> p h d", d=D),
                    )
                    yield

        def head_attn(h, filler, pending_rchain):
            """Attention for head h; pulls PE filler between steps.
            Issues its first two STs BEFORE running the previous head's
            r-chain (so ScalarE never starves at head boundaries), and
            returns its own r-chain as a closure for the next head."""
            t, hp = divmod(h, 2)
            hp *= D
            qT = qkT_sb[hp:hp + D, 2 * t]
            kT = qkT_sb[hp:hp + D, 2 * t + 1]
            # two 1-bank OT chunks (i cols 0:512 / 512:1024); a 3-slot pool
            # lets the next head's PV start while this head's r-chain runs
            ota = otps.tile([D + 1, 512], FP, tag="ot")
            otb = otps.tile([D + 1, 512], FP, tag="ot")
            ots = (ota, otb)

            def issue_st(j):
                st = stps.tile([P, N], FP, tag="st")
                ex = expp.tile([P, N], MMDT, tag="exp")
                for lo, hi in _chunks(N, 512):
                    nc.tensor.matmul(
                        st[:, lo:hi],
                        kT[:, j * P:(j + 1) * P],
                        qT[:, lo:hi],
                        start=True,
                        stop=True,
                    )
                nc.scalar.activation(
                    out=ex, in_=st, func=Exp, scale=float(SCALE),
                )
                return ex

            def issue_pv(j, ex):
                for c, (lo, hi) in enumerate(_chunks(N, 512)):
                    nc.tensor.matmul(
                        ots[c],
                        v_sb[:, j, h, :],
                        ex[:, lo:hi],
                        start=(j == 0),
                        stop=(j == NT - 1),
                    )

            def pull():
                try:
                    next(filler)
                except StopIteration:
                    pass

            exps = [issue_st(0), issue_st(1)]
            if pending_rchain is not None:
                pending_rchain()
            for j in range(NT):
                issue_pv(j, exps[j])
                pull()
                if j + 2 < NT:
                    exps.append(issue_st(j + 2))

            def rchain():
                # l rows (f32r) -> rank-1 ones⊗l broadcast into a PSUM slot
                # shared with the ST pool -> approx reciprocal (fp32) ->
                # normalize fused into the OT evacuation (bf16 out).
                la = rp.tile([1, 512], mybir.dt.float32r, tag="lrowa")
                lb_r = rp.tile([1, 512], mybir.dt.float32r, tag="lrowb")
                nc.vector.tensor_copy(out=la, in_=ota[D:D + 1, :])
                nc.vector.tensor_copy(out=lb_r, in_=otb[D:D + 1, :])
                pull()
                lbc = stps.tile([P, N], FP, tag="st")
                nc.tensor.matmul(lbc[:, 0:512], ones_f32r, la,
                                 start=True, stop=True)
                nc.tensor.matmul(lbc[:, 512:N], ones_f32r, lb_r,
                                 start=True, stop=True)
                rb_sb = rp.tile([P, N], FP, tag="rb")
                nc.vector.reciprocal_approx_fast(out=rb_sb, in_=lbc)
                nc.vector.tensor_tensor(
                    out=oT_sb[hp:hp + D, t, 0:512], in0=ota[0:D],
                    in1=rb_sb[0:D, 0:512], op=Mult,
                )
                nc.vector.tensor_tensor(
                    out=oT_sb[hp:hp + D, t, 512:N], in0=otb[0:D],
                    in1=rb_sb[0:D, 512:N], op=Mult,
                )

            return rchain

        # ---- interleaved pair loop ----
        def filler_for_pair(t):
            # 4 qk units per pair, spread over the 18+ pulls of two heads by
            # inserting pacing skips
            if t + 1 < NPAIR:
                for i, u in enumerate(gen_qk(t + 1)):
                    yield u
                    yield None  # pacing skip
                    yield None

        for _ in gen_qk(0):
            pass
        for _ in gen_v_all():
            pass
        pending = None
        for t in range(NPAIR):
            f = filler_for_pair(t)
            pending = head_attn(2 * t, f, pending)
            pending = head_attn(2 * t + 1, f, pending)
            for _ in f:
                pass
        pending()

      # ---------------- stage 3: output projection ----------------
      with (
            tc.tile_pool(name="s3w", bufs=1) as s3w,
            tc.tile_pool(name="s3y", bufs=2) as s3y,
            tc.tile_pool(name="s3ps", bufs=2, space="PSUM") as s3ps,
      ):
            wpT_sb = s3w.tile([P, KT, DIM], MMDT)
            wpr = wpT[:].rearrange("(t p) m -> t p m", p=P)
            for k in range(KT):
                nc.sync.dma_start(out=wpT_sb[:, k], in_=wpr[k])

            yr = y[:].rearrange("(i p) e -> i p e", p=P)
            for i in range(NT):
                ps = s3ps.tile([P, DIM], FP, tag="y")
                for lo, hi in _chunks(DIM, 512):
                    for k in range(KT):
                        nc.tensor.matmul(
                            ps[:, lo:hi],
                            oT_sb[:, k, i * P:(i + 1) * P],
                            wpT_sb[:, k, lo:hi],
                            start=(k == 0),
                            stop=(k == KT - 1),
                        )
                y_sb = s3y.tile([P, DIM], FP, tag="ysb")
                nc.vector.tensor_tensor(
                    out=y_sb, in0=ps, in1=bias_sb, op=Add,
                )
                nc.sync.dma_start(out=yr[i], in_=y_sb)


def prep_inputs(x, w_qkv, w_proj, b_proj):
    x = np.asarray(x, dtype=np.float32)
    w_qkv = np.asarray(w_qkv, dtype=np.float32)
    w_proj = np.asarray(w_proj, dtype=np.float32)
    b_proj = np.asarray(b_proj, dtype=np.float32)

    w_r = w_qkv.reshape(H, D, 3, DIM)  # rows ordered (h, d, qkv)
    wq = w_r[:, :, 0, :].reshape(DIM, DIM)  # rows (h, d)
    wk = w_r[:, :, 1, :].reshape(DIM, DIM)
    wv = w_r[:, :, 2, :].reshape(DIM, DIM)
    # pair-blocked qk: columns [q_t (128) | k_t (128)] for t = 0..5
    wqk_pairs = np.empty((2 * DIM, DIM), dtype=np.float32)
    for t in range(NPAIR):
        wqk_pairs[t * 256:t * 256 + P] = wq[t * P:(t + 1) * P]
        wqk_pairs[t * 256 + P:(t + 1) * 256] = wk[t * P:(t + 1) * P]
    wqkT = np.ascontiguousarray(wqk_pairs.T).astype(NP_MMDT)    # [768, 1536]
    wvT = np.ascontiguousarray(wv.T).astype(NP_MMDT)            # [768, 768]
    wpT = np.ascontiguousarray(w_proj.T).astype(NP_MMDT)        # [768, 768]
    xT = np.ascontiguousarray(x.transpose(0, 2, 1)).astype(NP_MMDT)
    bias = np.ascontiguousarray(b_proj.reshape(1, DIM))
    return xT, wqkT, wvT, wpT, bias


_NC = None
last_results = None


def get_nc():
    global _NC
    if _NC is None:
        _NC = build_nc()
    return _NC


def kernel(x, w_qkv, w_proj, b_proj):
    global last_results
    from concourse.bass_utils import run_bass_kernel_spmd

    nc = get_nc()
    xT, wqkT, wvT, wpT, bias = prep_inputs(x, w_qkv, w_proj, b_proj)
    in_maps = [
        {"xT": xT[c], "wqkT": wqkT, "wvT": wvT, "wpT": wpT, "bias": bias}
        for c in range(B)
    ]
    res = run_bass_kernel_spmd(nc, in_maps, core_ids=list(range(B)))
    last_results = res
    return np.stack([res.results[c]["y"] for c in range(B)], axis=0)


# revision 20
# speedup vs baseline: 1.0698x; 1.0698x over previous
"""Multi-head attention (B=8, N=1024, DIM=768, H=12) on 8 Trainium2 NeuronCores.

Sharding: pure data-parallel over the batch dimension — core c computes batch
element c end-to-end (qkv projection, softmax attention, output projection).
No collectives needed.

Numerics: matmul inputs in bf16 (x, weights, q/k, v, exp(P)) with fp32 PSUM
accumulation; softmax denominator, reciprocal, normalization and bias in
fp32. bf16 weight loads are FWL-accelerated and overlap matmuls via the PE
background weight buffer (fp32/f32r self-loading matmuls serialize a 213ns
LDWEIGHTS with every matmul - measured; that was the v2 bottleneck).

Schedule: stage-1 projection matmuls are interleaved INTO the attention loop
as PE filler so the tensor engine never idles while ScalarE computes exp (an
idle PE re-throttles to 1.2 GHz via HAM and doubles all matmul times).

  per head-pair t (heads 2t, 2t+1):
      qk pair-tile:  qkT[:, 2t], qkT[:, 2t+1] = [Wq_t; Wk_t] @ x^T
      v pair-slice:  v[:, :, 2t:2t+2] = x @ Wv_t^T   (+ ones column)
      per head: S^T[j,i] = k^T q (j on partitions), P = exp(S*scale) in one
      ACTIVATE per j-tile; PV accumulates OT_aug [d+1, i] with row 64 =
      softmax denominator l (ones column of v_aug).
      r-chain: l row -> SBUF (fp32) -> DRAM bounce -> broadcast DMA to all
      128 partitions -> reciprocal_approx_fast -> normalize fused into the
      PSUM->SBUF copy of OT (bf16 out).
  stage 3: y = OT^T @ WpT + bias (fp32)
"""

import os
import sys

for _p in ("/opt/trn_rl_repo",):
    if _p not in sys.path:
        sys.path.insert(0, _p)

import ml_dtypes
import numpy as np

import concourse.bass as bass
import concourse.tile as tile
from concourse import bacc, mybir

B, N, DIM, H = 8, 1024, 768, 12
D = DIM // H  # 64
SCALE = D ** -0.5
P = 128
KT = DIM // P        # 6 contraction tiles over dim
NT = N // P          # 8 tiles over sequence
NPAIR = H // 2       # 6 head pairs
FP = mybir.dt.float32
BF = mybir.dt.bfloat16
MMDT = BF
NP_MMDT = ml_dtypes.bfloat16


def _chunks(total, size):
    return [(lo, min(lo + size, total)) for lo in range(0, total, size)]


def build_nc():
    nc = bacc.Bacc(None, target_bir_lowering=False)
    xT = nc.dram_tensor("xT", [DIM, N], MMDT, kind="ExternalInput")
    # wqkT columns are pair-blocked: [q_t | k_t] of 128 cols each, t=0..5
    wqkT = nc.dram_tensor("wqkT", [DIM, 2 * DIM], MMDT, kind="ExternalInput")
    wvT = nc.dram_tensor("wvT", [DIM, DIM], MMDT, kind="ExternalInput")
    wpT = nc.dram_tensor("wpT", [DIM, DIM], MMDT, kind="ExternalInput")
    bias = nc.dram_tensor("bias", [1, DIM], FP, kind="ExternalInput")
    y = nc.dram_tensor("y", [N, DIM], FP, kind="ExternalOutput")

    with tile.TileContext(nc) as tc:
        with nc.allow_low_precision(reason="bf16 matmul inputs"):
            _body(tc, xT, wqkT, wvT, wpT, bias, y)
    nc.compile()
    return nc


def _body(tc, xT, wqkT, wvT, wpT, bias, y):
    nc = tc.nc
    Exp = mybir.ActivationFunctionType.Exp
    Mult = mybir.AluOpType.mult
    Add = mybir.AluOpType.add

    from contextlib import ExitStack
    with tc.tile_pool(name="persist", bufs=1) as persist:
      with ExitStack() as s12:
        s1w = s12.enter_context(tc.tile_pool(name="s1w", bufs=1))
        expp = s12.enter_context(tc.tile_pool(name="expp", bufs=3))
        rp = s12.enter_context(tc.tile_pool(name="rp", bufs=2))
        s1ps = s12.enter_context(tc.tile_pool(name="s1ps", bufs=1, space="PSUM"))
        stps = s12.enter_context(tc.tile_pool(name="stps", bufs=2, space="PSUM"))
        otps = s12.enter_context(tc.tile_pool(name="otps", bufs=3, space="PSUM"))

        # qkT_sb tile index 2t = q of pair t, 2t+1 = k of pair t; rows (h%2,d)
        qkT_sb = persist.tile([P, 2 * KT, N], MMDT)     # 24 KB/part
        v_sb = persist.tile([P, NT, H, D + 1], MMDT)    # 12.7 KB/part
        oT_sb = persist.tile([P, KT, N], MMDT)          # 12 KB/part
        bias_sb = persist.tile([P, DIM], FP)            # 3 KB/part
        y_acc = persist.tile([P, NT, DIM], FP)          # 24 KB/part
        ones_f32r = persist.tile([1, P], mybir.dt.float32r)
        ones_stg = persist.tile([1, P], FP)
        nc.sync.dma_start(out=bias_sb, in_=bias[:].to_broadcast((P, DIM)))
        nc.vector.memset(v_sb[:, :, :, D], 1.0)
        nc.vector.memset(ones_stg, 1.0)
        nc.vector.tensor_copy(out=ones_f32r, in_=ones_stg)

        xT_sb = s1w.tile([P, KT, N], MMDT)              # 12 KB/part
        wqkT_sb = s1w.tile([P, KT, 2 * DIM], MMDT)      # 18 KB/part
        wvT_sb = s1w.tile([P, KT, DIM], MMDT)           # 9 KB/part
        wpT_sb = s1w.tile([P, KT, DIM], MMDT)           # 9 KB/part

        xTr = xT[:].rearrange("(t p) n -> t p n", p=P)
        wqkr = wqkT[:].rearrange("(t p) m -> t p m", p=P)
        wvr = wvT[:].rearrange("(t p) m -> t p m", p=P)
        wpr = wpT[:].rearrange("(t p) m -> t p m", p=P)

        # DMA order: x first (everything needs it), then per-pair slices of
        # wqk / wv so pair-0 unblocks earliest.
        for k in range(KT):
            nc.sync.dma_start(out=xT_sb[:, k], in_=xTr[k])
            nc.sync.dma_start(out=wqkT_sb[:, k, 0:256], in_=wqkr[k][:, 0:256])
            nc.sync.dma_start(out=wvT_sb[:, k, 0:P], in_=wvr[k][:, 0:P])
        for t in range(1, NPAIR):
            for k in range(KT):
                nc.sync.dma_start(
                    out=wqkT_sb[:, k, t * 256:(t + 1) * 256],
                    in_=wqkr[k][:, t * 256:(t + 1) * 256],
                )
            for k in range(KT):
                nc.sync.dma_start(
                    out=wvT_sb[:, k, t * P:(t + 1) * P],
                    in_=wvr[k][:, t * P:(t + 1) * P],
                )
        for k in range(KT):
            nc.sync.dma_start(out=wpT_sb[:, k], in_=wpr[k])

        # ---- PE work generators (filler units of ~0.5-1.3us of matmuls) ----
        def gen_qk(t):
            """qk pair-tile t -> qkT_sb[:, 2t] (q) and [:, 2t+1] (k)."""
            for which in range(2):
                for lo, hi in _chunks(N, 512):
                    ps = s1ps.tile([P, 512], FP, tag="s1")
                    for k in range(KT):
                        nc.tensor.matmul(
                            ps,
                            wqkT_sb[:, k, t * 256 + which * P:
                                    t * 256 + (which + 1) * P],
                            xT_sb[:, k, lo:hi],
                            start=(k == 0),
                            stop=(k == KT - 1),
                        )
                    nc.vector.tensor_copy(
                        out=qkT_sb[:, 2 * t + which, lo:hi], in_=ps)
                    yield

        def gen_v(t):
            """v pair-slice t -> v_sb[:, :, 2t:2t+2, 0:D]."""
            for half in range(2):
                ps = s1ps.tile([P, 512], FP, tag="s1")
                for jj in range(4):
                    j = half * 4 + jj
                    for k in range(KT):
                        nc.tensor.matmul(
                            ps[:, jj * P:(jj + 1) * P],
                            xT_sb[:, k, j * P:(j + 1) * P],
                            wvT_sb[:, k, t * P:(t + 1) * P],
                            start=(k == 0),
                            stop=(k == KT - 1),
                        )
                    yield
                nc.vector.tensor_copy(
                    out=v_sb[:, half * 4:(half + 1) * 4, 2 * t:2 * t + 2, 0:D],
                    in_=ps.rearrange("p (j g d) -> p j g d", g=2, d=D),
                )

        def head_attn(h, filler, pending_rchain):
            """Attention for head h; pulls PE filler between steps.
            Issues its first two STs BEFORE running the previous head's
            r-chain (so ScalarE never starves at head boundaries), and
            returns its own r-chain as a closure for the next head."""
            t, hp = divmod(h, 2)
            hp *= D
            qT = qkT_sb[hp:hp + D, 2 * t]
            kT = qkT_sb[hp:hp + D, 2 * t + 1]
            # two 1-bank OT chunks (i cols 0:512 / 512:1024); a 3-slot pool
            # lets the next head's PV start while this head's r-chain runs
            ota = otps.tile([D + 1, 512], FP, tag="ot")
            otb = otps.tile([D + 1, 512], FP, tag="ot")
            ots = (ota, otb)

            def issue_st(j):
                st = stps.tile([P, N], FP, tag="st")
                ex = expp.tile([P, N], MMDT, tag="exp")
                for lo, hi in _chunks(N, 512):
                    nc.tensor.matmul(
                        st[:, lo:hi],
                        kT[:, j * P:(j + 1) * P],
                        qT[:, lo:hi],
                        start=True,
                        stop=True,
                    )
                nc.scalar.activation(
                    out=ex, in_=st, func=Exp, scale=float(SCALE),
                )
                return ex

            def issue_pv(j, ex):
                for c, (lo, hi) in enumerate(_chunks(N, 512)):
                    nc.tensor.matmul(
                        ots[c],
                        v_sb[:, j, h, :],
                        ex[:, lo:hi],
                        start=(j == 0),
                        stop=(j == NT - 1),
                    )

            def pull():
                try:
                    next(filler)
                except StopIteration:
                    pass

            exps = [issue_st(0), issue_st(1)]
            if pending_rchain is not None:
                pending_rchain()
            for j in range(NT):
                issue_pv(j, exps[j])
                pull()
                if j + 2 < NT:
                    exps.append(issue_st(j + 2))

            def rchain():
                # l rows (f32r) -> rank-1 ones⊗l broadcast into a PSUM slot
                # shared with the ST pool -> approx reciprocal (fp32) ->
                # normalize fused into the OT evacuation (bf16 out).
                la = rp.tile([1, 512], mybir.dt.float32r, tag="lrowa")
                lb_r = rp.tile([1, 512], mybir.dt.float32r, tag="lrowb")
                nc.vector.tensor_copy(out=la, in_=ota[D:D + 1, :])
                nc.vector.tensor_copy(out=lb_r, in_=otb[D:D + 1, :])
                pull()
                lbc = stps.tile([P, N], FP, tag="st")
                nc.tensor.matmul(lbc[:, 0:512], ones_f32r, la,
                                 start=True, stop=True)
                nc.tensor.matmul(lbc[:, 512:N], ones_f32r, lb_r,
                                 start=True, stop=True)
                rb_sb = rp.tile([P, N], FP, tag="rb")
                nc.vector.reciprocal_approx_fast(out=rb_sb, in_=lbc)
                nc.vector.tensor_tensor(
                    out=oT_sb[hp:hp + D, t, 0:512], in0=ota[0:D],
                    in1=rb_sb[0:D, 0:512], op=Mult,
                )
                nc.vector.tensor_tensor(
                    out=oT_sb[hp:hp + D, t, 512:N], in0=otb[0:D],
                    in1=rb_sb[0:D, 512:N], op=Mult,
                )

            return rchain

        def gen_proj_partial():
            """Output-projection contributions of k-tiles 0..4 (pairs 0-4),
            SBUF-accumulated into y_acc; runs as PE filler during pair 5 so
            only the thin k=5 pass remains after the last head."""
            for i in range(NT):
                for lo, hi in _chunks(DIM, 512):
                    ps = s1ps.tile([P, 512], FP, tag="s1")
                    for k in range(KT - 1):
                        nc.tensor.matmul(
                            ps[:, 0:hi - lo],
                            oT_sb[:, k, i * P:(i + 1) * P],
                            wpT_sb[:, k, lo:hi],
                            start=(k == 0),
                            stop=(k == KT - 2),
                        )
                    nc.vector.tensor_copy(
                        out=y_acc[:, i, lo:hi], in_=ps[:, 0:hi - lo])
                    yield

        # ---- interleaved pair loop ----
        def filler_for_pair(t):
            # spread filler units over the 18+ pulls of two heads by
            # inserting pacing skips
            if t + 1 < NPAIR:
                def units():
                    yield from gen_qk(t + 1)
                    yield from gen_v(t + 1)
                for i, u in enumerate(units()):
                    yield u
                    if i % 4 == 3:
                        yield None  # pacing skip
            else:
                for u in gen_proj_partial():
                    yield u

        for _ in gen_qk(0):
            pass
        for _ in gen_v(0):
            pass
        pending = None
        for t in range(NPAIR):
            f = filler_for_pair(t)
            pending = head_attn(2 * t, f, pending)
            pending = head_attn(2 * t + 1, f, pending)
            for _ in f:
                pass
        pending()

      # -------- stage 3: last projection k-tile (5) + combine --------
      with (
            tc.tile_pool(name="s3y", bufs=4) as s3y,
            tc.tile_pool(name="s3ps", bufs=2, space="PSUM") as s3ps,
      ):
            yr = y[:].rearrange("(i p) e -> i p e", p=P)
            for i in range(NT):
                ps = s3ps.tile([P, DIM], FP, tag="y")
                for lo, hi in _chunks(DIM, 512):
                    nc.tensor.matmul(
                        ps[:, lo:hi],
                        oT_sb[:, KT - 1, i * P:(i + 1) * P],
                        wpT_sb[:, KT - 1, lo:hi],
                        start=True,
                        stop=True,
                    )
                y_sb = s3y.tile([P, DIM], FP, tag="ysb")
                nc.vector.tensor_tensor(
                    out=y_sb, in0=ps, in1=y_acc[:, i], op=Add,
                )
                nc.vector.tensor_tensor(
                    out=y_sb, in0=y_sb, in1=bias_sb, op=Add,
                )
                nc.sync.dma_start(out=yr[i], in_=y_sb)


def prep_inputs(x, w_qkv, w_proj, b_proj):
    x = np.asarray(x, dtype=np.float32)
    w_qkv = np.asarray(w_qkv, dtype=np.float32)
    w_proj = np.asarray(w_proj, dtype=np.float32)
    b_proj = np.asarray(b_proj, dtype=np.float32)

    w_r = w_qkv.reshape(H, D, 3, DIM)  # rows ordered (h, d, qkv)
    wq = w_r[:, :, 0, :].reshape(DIM, DIM)  # rows (h, d)
    wk = w_r[:, :, 1, :].reshape(DIM, DIM)
    wv = w_r[:, :, 2, :].reshape(DIM, DIM)
    # pair-blocked qk: columns [q_t (128) | k_t (128)] for t = 0..5
    wqk_pairs = np.empty((2 * DIM, DIM), dtype=np.float32)
    for t in range(NPAIR):
        wqk_pairs[t * 256:t * 256 + P] = wq[t * P:(t + 1) * P]
        wqk_pairs[t * 256 + P:(t + 1) * 256] = wk[t * P:(t + 1) * P]
    wqkT = np.ascontiguousarray(wqk_pairs.T).astype(NP_MMDT)    # [768, 1536]
    wvT = np.ascontiguousarray(wv.T).astype(NP_MMDT)            # [768, 768]
    wpT = np.ascontiguousarray(w_proj.T).astype(NP_MMDT)        # [768, 768]
    xT = np.ascontiguousarray(x.transpose(0, 2, 1)).astype(NP_MMDT)
    bias = np.ascontiguousarray(b_proj.reshape(1, DIM))
    return xT, wqkT, wvT, wpT, bias


_NC = None
last_results = None


def get_nc():
    global _NC
    if _NC is None:
        _NC = build_nc()
    return _NC


def kernel(x, w_qkv, w_proj, b_proj):
    global last_results
    from concourse.bass_utils import run_bass_kernel_spmd

    nc = get_nc()
    xT, wqkT, wvT, wpT, bias = prep_inputs(x, w_qkv, w_proj, b_proj)
    in_maps = [
        {"xT": xT[c], "wqkT": wqkT, "wvT": wvT, "wpT": wpT, "bias": bias}
        for c in range(B)
    ]
    res = run_bass_kernel_spmd(nc, in_maps, core_ids=list(range(B)))
    last_results = res
    return np.stack([res.results[c]["y"] for c in range(B)], axis=0)


# revision 21
# speedup vs baseline: 1.0729x; 1.0029x over previous
"""Multi-head attention (B=8, N=1024, DIM=768, H=12) on 8 Trainium2 NeuronCores.

Sharding: pure data-parallel over the batch dimension — core c computes batch
element c end-to-end (qkv projection, softmax attention, output projection).
No collectives needed.

Numerics: matmul inputs in bf16 (x, weights, q/k, v, exp(P)) with fp32 PSUM
accumulation; softmax denominator, reciprocal, normalization and bias in
fp32. bf16 weight loads are FWL-accelerated and overlap matmuls via the PE
background weight buffer (fp32/f32r self-loading matmuls serialize a 213ns
LDWEIGHTS with every matmul - measured; that was the v2 bottleneck).

Schedule: stage-1 projection matmuls are interleaved INTO the attention loop
as PE filler so the tensor engine never idles while ScalarE computes exp (an
idle PE re-throttles to 1.2 GHz via HAM and doubles all matmul times).

  per head-pair t (heads 2t, 2t+1):
      qk pair-tile:  qkT[:, 2t], qkT[:, 2t+1] = [Wq_t; Wk_t] @ x^T
      v pair-slice:  v[:, :, 2t:2t+2] = x @ Wv_t^T   (+ ones column)
      per head: S^T[j,i] = k^T q (j on partitions), P = exp(S*scale) in one
      ACTIVATE per j-tile; PV accumulates OT_aug [d+1, i] with row 64 =
      softmax denominator l (ones column of v_aug).
      r-chain: l row -> SBUF (fp32) -> DRAM bounce -> broadcast DMA to all
      128 partitions -> reciprocal_approx_fast -> normalize fused into the
      PSUM->SBUF copy of OT (bf16 out).
  stage 3: y = OT^T @ WpT + bias (fp32)
"""

import os
import sys

for _p in ("/opt/trn_rl_repo",):
    if _p not in sys.path:
        sys.path.insert(0, _p)

import ml_dtypes
import numpy as np

import concourse.bass as bass
import concourse.tile as tile
from concourse import bacc, mybir

B, N, DIM, H = 8, 1024, 768, 12
D = DIM // H  # 64
SCALE = D ** -0.5
P = 128
KT = DIM // P        # 6 contraction tiles over dim
NT = N // P          # 8 tiles over sequence
NPAIR = H // 2       # 6 head pairs
FP = mybir.dt.float32
BF = mybir.dt.bfloat16
MMDT = BF
NP_MMDT = ml_dtypes.bfloat16


def _chunks(total, size):
    return [(lo, min(lo + size, total)) for lo in range(0, total, size)]


def build_nc():
    nc = bacc.Bacc(None, target_bir_lowering=False)
    xT = nc.dram_tensor("xT", [DIM, N], MMDT, kind="ExternalInput")
    # wqkT columns are pair-blocked: [q_t | k_t] of 128 cols each, t=0..5
    wqkT = nc.dram_tensor("wqkT", [DIM, 2 * DIM], MMDT, kind="ExternalInput")
    wvT = nc.dram_tensor("wvT", [DIM, DIM], MMDT, kind="ExternalInput")
    wpT = nc.dram_tensor("wpT", [DIM, DIM], MMDT, kind="ExternalInput")
    bias = nc.dram_tensor("bias", [1, DIM], FP, kind="ExternalInput")
    y = nc.dram_tensor("y", [N, DIM], FP, kind="ExternalOutput")

    with tile.TileContext(nc) as tc:
        with nc.allow_low_precision(reason="bf16 matmul inputs"):
            _body(tc, xT, wqkT, wvT, wpT, bias, y)
    nc.compile()
    return nc


def _body(tc, xT, wqkT, wvT, wpT, bias, y):
    nc = tc.nc
    Exp = mybir.ActivationFunctionType.Exp
    Mult = mybir.AluOpType.mult
    Add = mybir.AluOpType.add

    from contextlib import ExitStack
    with tc.tile_pool(name="persist", bufs=1) as persist:
      with ExitStack() as s12:
        s1w = s12.enter_context(tc.tile_pool(name="s1w", bufs=1))
        expp = s12.enter_context(tc.tile_pool(name="expp", bufs=3))
        rp = s12.enter_context(tc.tile_pool(name="rp", bufs=2))
        s1ps = s12.enter_context(tc.tile_pool(name="s1ps", bufs=1, space="PSUM"))
        stps = s12.enter_context(tc.tile_pool(name="stps", bufs=2, space="PSUM"))
        otps = s12.enter_context(tc.tile_pool(name="otps", bufs=3, space="PSUM"))

        # qkT_sb tile index 2t = q of pair t, 2t+1 = k of pair t; rows (h%2,d)
        qkT_sb = persist.tile([P, 2 * KT, N], MMDT)     # 24 KB/part
        v_sb = persist.tile([P, NT, H, D + 1], MMDT)    # 12.7 KB/part
        oT_sb = persist.tile([P, KT, N], MMDT)          # 12 KB/part
        bias_sb = persist.tile([P, DIM], FP)            # 3 KB/part
        y_acc = persist.tile([P, NT, DIM], FP)          # 24 KB/part
        ones_f32r = persist.tile([1, P], mybir.dt.float32r)
        ones_stg = persist.tile([1, P], FP)
        nc.sync.dma_start(out=bias_sb, in_=bias[:].to_broadcast((P, DIM)))
        nc.vector.memset(v_sb[:, :, :, D], 1.0)
        nc.vector.memset(ones_stg, 1.0)
        nc.vector.tensor_copy(out=ones_f32r, in_=ones_stg)

        xT_sb = s1w.tile([P, KT, N], MMDT)              # 12 KB/part
        wqkT_sb = s1w.tile([P, KT, 2 * DIM], MMDT)      # 18 KB/part
        wvT_sb = s1w.tile([P, KT, DIM], MMDT)           # 9 KB/part
        wpT_sb = s1w.tile([P, KT, DIM], MMDT)           # 9 KB/part

        xTr = xT[:].rearrange("(t p) n -> t p n", p=P)
        wqkr = wqkT[:].rearrange("(t p) m -> t p m", p=P)
        wvr = wvT[:].rearrange("(t p) m -> t p m", p=P)
        wpr = wpT[:].rearrange("(t p) m -> t p m", p=P)

        # Inputs split across BOTH HWDGE trigger queues (sync + scalar) for
        # parallel DMA rings; batched 3D descriptors. x on sync; pair-0
        # weights on scalar so qk(0)/v(0) unblock at ~x-landing time.
        nc.sync.dma_start(out=xT_sb, in_=xTr.rearrange("t p n -> p t n"))
        wqk_t = wqkr.rearrange("t p m -> p t m")
        wv_t = wvr.rearrange("t p m -> p t m")
        nc.scalar.dma_start(out=wqkT_sb[:, :, 0:256], in_=wqk_t[:, :, 0:256])
        nc.scalar.dma_start(out=wvT_sb[:, :, 0:P], in_=wv_t[:, :, 0:P])
        for t in range(1, NPAIR):
            eng = nc.sync if t % 2 else nc.scalar
            eng.dma_start(
                out=wqkT_sb[:, :, t * 256:(t + 1) * 256],
                in_=wqk_t[:, :, t * 256:(t + 1) * 256],
            )
            eng.dma_start(
                out=wvT_sb[:, :, t * P:(t + 1) * P],
                in_=wv_t[:, :, t * P:(t + 1) * P],
            )
        nc.scalar.dma_start(out=wpT_sb, in_=wpr.rearrange("t p m -> p t m"))

        # ---- PE work generators (filler units of ~0.5-1.3us of matmuls) ----
        def gen_qk(t):
            """qk pair-tile t -> qkT_sb[:, 2t] (q) and [:, 2t+1] (k)."""
            for which in range(2):
                for lo, hi in _chunks(N, 512):
                    ps = s1ps.tile([P, 512], FP, tag="s1")
                    for k in range(KT):
                        nc.tensor.matmul(
                            ps,
                            wqkT_sb[:, k, t * 256 + which * P:
                                    t * 256 + (which + 1) * P],
                            xT_sb[:, k, lo:hi],
                            start=(k == 0),
                            stop=(k == KT - 1),
                        )
                    nc.vector.tensor_copy(
                        out=qkT_sb[:, 2 * t + which, lo:hi], in_=ps)
                    yield

        def gen_v(t):
            """v pair-slice t -> v_sb[:, :, 2t:2t+2, 0:D]."""
            for half in range(2):
                ps = s1ps.tile([P, 512], FP, tag="s1")
                for jj in range(4):
                    j = half * 4 + jj
                    for k in range(KT):
                        nc.tensor.matmul(
                            ps[:, jj * P:(jj + 1) * P],
                            xT_sb[:, k, j * P:(j + 1) * P],
                            wvT_sb[:, k, t * P:(t + 1) * P],
                            start=(k == 0),
                            stop=(k == KT - 1),
                        )
                    yield
                nc.vector.tensor_copy(
                    out=v_sb[:, half * 4:(half + 1) * 4, 2 * t:2 * t + 2, 0:D],
                    in_=ps.rearrange("p (j g d) -> p j g d", g=2, d=D),
                )

        def head_attn(h, filler, pending_rchain):
            """Attention for head h; pulls PE filler between steps.
            Issues its first two STs BEFORE running the previous head's
            r-chain (so ScalarE never starves at head boundaries), and
            returns its own r-chain as a closure for the next head."""
            t, hp = divmod(h, 2)
            hp *= D
            qT = qkT_sb[hp:hp + D, 2 * t]
            kT = qkT_sb[hp:hp + D, 2 * t + 1]
            # two 1-bank OT chunks (i cols 0:512 / 512:1024); a 3-slot pool
            # lets the next head's PV start while this head's r-chain runs
            ota = otps.tile([D + 1, 512], FP, tag="ot")
            otb = otps.tile([D + 1, 512], FP, tag="ot")
            ots = (ota, otb)

            def issue_st(j):
                st = stps.tile([P, N], FP, tag="st")
                ex = expp.tile([P, N], MMDT, tag="exp")
                for lo, hi in _chunks(N, 512):
                    nc.tensor.matmul(
                        st[:, lo:hi],
                        kT[:, j * P:(j + 1) * P],
                        qT[:, lo:hi],
                        start=True,
                        stop=True,
                    )
                nc.scalar.activation(
                    out=ex, in_=st, func=Exp, scale=float(SCALE),
                )
                return ex

            def issue_pv(j, ex):
                for c, (lo, hi) in enumerate(_chunks(N, 512)):
                    nc.tensor.matmul(
                        ots[c],
                        v_sb[:, j, h, :],
                        ex[:, lo:hi],
                        start=(j == 0),
                        stop=(j == NT - 1),
                    )

            def pull():
                try:
                    next(filler)
                except StopIteration:
                    pass

            exps = [issue_st(0), issue_st(1)]
            if pending_rchain is not None:
                pending_rchain()
            for j in range(NT):
                issue_pv(j, exps[j])
                pull()
                if j + 2 < NT:
                    exps.append(issue_st(j + 2))

            def rchain():
                # l rows (f32r) -> rank-1 ones⊗l broadcast into a PSUM slot
                # shared with the ST pool -> approx reciprocal (fp32) ->
                # normalize fused into the OT evacuation (bf16 out).
                la = rp.tile([1, 512], mybir.dt.float32r, tag="lrowa")
                lb_r = rp.tile([1, 512], mybir.dt.float32r, tag="lrowb")
                nc.vector.tensor_copy(out=la, in_=ota[D:D + 1, :])
                nc.vector.tensor_copy(out=lb_r, in_=otb[D:D + 1, :])
                pull()
                lbc = stps.tile([P, N], FP, tag="st")
                nc.tensor.matmul(lbc[:, 0:512], ones_f32r, la,
                                 start=True, stop=True)
                nc.tensor.matmul(lbc[:, 512:N], ones_f32r, lb_r,
                                 start=True, stop=True)
                rb_sb = rp.tile([P, N], FP, tag="rb")
                nc.vector.reciprocal_approx_fast(out=rb_sb, in_=lbc)
                nc.vector.tensor_tensor(
                    out=oT_sb[hp:hp + D, t, 0:512], in0=ota[0:D],
                    in1=rb_sb[0:D, 0:512], op=Mult,
                )
                nc.vector.tensor_tensor(
                    out=oT_sb[hp:hp + D, t, 512:N], in0=otb[0:D],
                    in1=rb_sb[0:D, 512:N], op=Mult,
                )

            return rchain

        def gen_proj_partial():
            """Output-projection contributions of k-tiles 0..4 (pairs 0-4),
            SBUF-accumulated into y_acc; runs as PE filler during pair 5 so
            only the thin k=5 pass remains after the last head."""
            for i in range(NT):
                for lo, hi in _chunks(DIM, 512):
                    ps = s1ps.tile([P, 512], FP, tag="s1")
                    for k in range(KT - 1):
                        nc.tensor.matmul(
                            ps[:, 0:hi - lo],
                            oT_sb[:, k, i * P:(i + 1) * P],
                            wpT_sb[:, k, lo:hi],
                            start=(k == 0),
                            stop=(k == KT - 2),
                        )
                    nc.vector.tensor_copy(
                        out=y_acc[:, i, lo:hi], in_=ps[:, 0:hi - lo])
                    yield

        # ---- interleaved pair loop ----
        def filler_for_pair(t):
            # spread filler units over the 18+ pulls of two heads by
            # inserting pacing skips
            if t + 1 < NPAIR:
                def units():
                    yield from gen_qk(t + 1)
                    yield from gen_v(t + 1)
                for i, u in enumerate(units()):
                    yield u
                    if i % 4 == 3:
                        yield None  # pacing skip
            else:
                for u in gen_proj_partial():
                    yield u

        for _ in gen_qk(0):
            pass
        pending = None
        for t in range(NPAIR):
            if t == 0:
                def f0():
                    yield from gen_v(0)
                    yield from filler_for_pair(0)
                f = f0()
            else:
                f = filler_for_pair(t)
            pending = head_attn(2 * t, f, pending)
            pending = head_attn(2 * t + 1, f, pending)
            for _ in f:
                pass
        pending()

      # -------- stage 3: last projection k-tile (5) + combine --------
      with (
            tc.tile_pool(name="s3y", bufs=4) as s3y,
            tc.tile_pool(name="s3ps", bufs=2, space="PSUM") as s3ps,
      ):
            yr = y[:].rearrange("(i p) e -> i p e", p=P)
            for i in range(NT):
                ps = s3ps.tile([P, DIM], FP, tag="y")
                for lo, hi in _chunks(DIM, 512):
                    nc.tensor.matmul(
                        ps[:, lo:hi],
                        oT_sb[:, KT - 1, i * P:(i + 1) * P],
                        wpT_sb[:, KT - 1, lo:hi],
                        start=True,
                        stop=True,
                    )
                y_sb = s3y.tile([P, DIM], FP, tag="ysb")
                nc.vector.tensor_tensor(
                    out=y_sb, in0=ps, in1=y_acc[:, i], op=Add,
                )
                nc.vector.tensor_tensor(
                    out=y_sb, in0=y_sb, in1=bias_sb, op=Add,
                )
                nc.sync.dma_start(out=yr[i], in_=y_sb)


def prep_inputs(x, w_qkv, w_proj, b_proj):
    x = np.asarray(x, dtype=np.float32)
    w_qkv = np.asarray(w_qkv, dtype=np.float32)
    w_proj = np.asarray(w_proj, dtype=np.float32)
    b_proj = np.asarray(b_proj, dtype=np.float32)

    w_r = w_qkv.reshape(H, D, 3, DIM)  # rows ordered (h, d, qkv)
    wq = w_r[:, :, 0, :].reshape(DIM, DIM)  # rows (h, d)
    wk = w_r[:, :, 1, :].reshape(DIM, DIM)
    wv = w_r[:, :, 2, :].reshape(DIM, DIM)
    # pair-blocked qk: columns [q_t (128) | k_t (128)] for t = 0..5
    wqk_pairs = np.empty((2 * DIM, DIM), dtype=np.float32)
    for t in range(NPAIR):
        wqk_pairs[t * 256:t * 256 + P] = wq[t * P:(t + 1) * P]
        wqk_pairs[t * 256 + P:(t + 1) * 256] = wk[t * P:(t + 1) * P]
    wqkT = np.ascontiguousarray(wqk_pairs.T).astype(NP_MMDT)    # [768, 1536]
    wvT = np.ascontiguousarray(wv.T).astype(NP_MMDT)            # [768, 768]
    wpT = np.ascontiguousarray(w_proj.T).astype(NP_MMDT)        # [768, 768]
    xT = np.ascontiguousarray(x.transpose(0, 2, 1)).astype(NP_MMDT)
    bias = np.ascontiguousarray(b_proj.reshape(1, DIM))
    return xT, wqkT, wvT, wpT, bias


_NC = None
last_results = None


def get_nc():
    global _NC
    if _NC is None:
        _NC = build_nc()
    return _NC


def kernel(x, w_qkv, w_proj, b_proj):
    global last_results
    from concourse.bass_utils import run_bass_kernel_spmd

    nc = get_nc()
    xT, wqkT, wvT, wpT, bias = prep_inputs(x, w_qkv, w_proj, b_proj)
    in_maps = [
        {"xT": xT[c], "wqkT": wqkT, "wvT": wvT, "wpT": wpT, "bias": bias}
        for c in range(B)
    ]
    res = run_bass_kernel_spmd(nc, in_maps, core_ids=list(range(B)))
    last_results = res
    return np.stack([res.results[c]["y"] for c in range(B)], axis=0)


# revision 22
# speedup vs baseline: 1.0856x; 1.0118x over previous
"""Multi-head attention (B=8, N=1024, DIM=768, H=12) on 8 Trainium2 NeuronCores.

Sharding: pure data-parallel over the batch dimension — core c computes batch
element c end-to-end (qkv projection, softmax attention, output projection).
No collectives needed.

Numerics: matmul inputs in bf16 (x, weights, q/k, v, exp(P)) with fp32 PSUM
accumulation; softmax denominator, reciprocal, normalization and bias in
fp32. bf16 weight loads are FWL-accelerated and overlap matmuls via the PE
background weight buffer (fp32/f32r self-loading matmuls serialize a 213ns
LDWEIGHTS with every matmul - measured; that was the v2 bottleneck).

Schedule: stage-1 projection matmuls are interleaved INTO the attention loop
as PE filler so the tensor engine never idles while ScalarE computes exp (an
idle PE re-throttles to 1.2 GHz via HAM and doubles all matmul times).

  per head-pair t (heads 2t, 2t+1):
      qk pair-tile:  qkT[:, 2t], qkT[:, 2t+1] = [Wq_t; Wk_t] @ x^T
      v pair-slice:  v[:, :, 2t:2t+2] = x @ Wv_t^T   (+ ones column)
      per head: S^T[j,i] = k^T q (j on partitions), P = exp(S*scale) in one
      ACTIVATE per j-tile; PV accumulates OT_aug [d+1, i] with row 64 =
      softmax denominator l (ones column of v_aug).
      r-chain: l row -> SBUF (fp32) -> DRAM bounce -> broadcast DMA to all
      128 partitions -> reciprocal_approx_fast -> normalize fused into the
      PSUM->SBUF copy of OT (bf16 out).
  stage 3: y = OT^T @ WpT + bias (fp32)
"""

import os
import sys

for _p in ("/opt/trn_rl_repo",):
    if _p not in sys.path:
        sys.path.insert(0, _p)

import ml_dtypes
import numpy as np

import concourse.bass as bass
import concourse.tile as tile
from concourse import bacc, mybir

B, N, DIM, H = 8, 1024, 768, 12
D = DIM // H  # 64
SCALE = D ** -0.5
P = 128
KT = DIM // P        # 6 contraction tiles over dim
NT = N // P          # 8 tiles over sequence
NPAIR = H // 2       # 6 head pairs
FP = mybir.dt.float32
BF = mybir.dt.bfloat16
MMDT = BF
NP_MMDT = ml_dtypes.bfloat16


def _chunks(total, size):
    return [(lo, min(lo + size, total)) for lo in range(0, total, size)]


def build_nc():
    nc = bacc.Bacc(None, target_bir_lowering=False)
    xT = nc.dram_tensor("xT", [DIM, N], MMDT, kind="ExternalInput")
    # wqkT columns are pair-blocked: [q_t | k_t] of 128 cols each, t=0..5
    wqkT = nc.dram_tensor("wqkT", [DIM, 2 * DIM], MMDT, kind="ExternalInput")
    wvT = nc.dram_tensor("wvT", [DIM, DIM], MMDT, kind="ExternalInput")
    wpT = nc.dram_tensor("wpT", [DIM, DIM], MMDT, kind="ExternalInput")
    bias = nc.dram_tensor("bias", [1, DIM], FP, kind="ExternalInput")
    y = nc.dram_tensor("y", [N, DIM], FP, kind="ExternalOutput")

    with tile.TileContext(nc) as tc:
        with nc.allow_low_precision(reason="bf16 matmul inputs"):
            _body(tc, xT, wqkT, wvT, wpT, bias, y)
    nc.compile()
    return nc


def _body(tc, xT, wqkT, wvT, wpT, bias, y):
    nc = tc.nc
    Exp = mybir.ActivationFunctionType.Exp
    Mult = mybir.AluOpType.mult
    Add = mybir.AluOpType.add

    from contextlib import ExitStack
    with tc.tile_pool(name="persist", bufs=1) as persist:
      with ExitStack() as s12:
        s1w = s12.enter_context(tc.tile_pool(name="s1w", bufs=1))
        expp = s12.enter_context(tc.tile_pool(name="expp", bufs=3))
        rp = s12.enter_context(tc.tile_pool(name="rp", bufs=2))
        s1ps = s12.enter_context(tc.tile_pool(name="s1ps", bufs=1, space="PSUM"))
        stps = s12.enter_context(tc.tile_pool(name="stps", bufs=2, space="PSUM"))
        otps = s12.enter_context(tc.tile_pool(name="otps", bufs=3, space="PSUM"))

        # qkT_sb tile index 2t = q of pair t, 2t+1 = k of pair t; rows (h%2,d)
        qkT_sb = persist.tile([P, 2 * KT, N], MMDT)     # 24 KB/part
        v_sb = persist.tile([P, NT, H, D + 1], MMDT)    # 12.7 KB/part
        oT_sb = persist.tile([P, KT, N], MMDT)          # 12 KB/part
        bias_sb = persist.tile([P, DIM], FP)            # 3 KB/part
        y_acc = persist.tile([P, NT, DIM], FP)          # 24 KB/part
        ones_f32r = persist.tile([1, P], mybir.dt.float32r)
        ones_stg = persist.tile([1, P], FP)
        nc.sync.dma_start(out=bias_sb, in_=bias[:].to_broadcast((P, DIM)))
        nc.vector.memset(v_sb[:, :, :, D], 1.0)
        nc.vector.memset(ones_stg, 1.0)
        nc.vector.tensor_copy(out=ones_f32r, in_=ones_stg)

        xT_sb = s1w.tile([P, KT, N], MMDT)              # 12 KB/part
        wqkT_sb = s1w.tile([P, KT, 2 * DIM], MMDT)      # 18 KB/part
        wvT_sb = s1w.tile([P, KT, DIM], MMDT)           # 9 KB/part
        wpT_sb = s1w.tile([P, KT, DIM], MMDT)           # 9 KB/part

        xTr = xT[:].rearrange("(t p) n -> t p n", p=P)
        wqkr = wqkT[:].rearrange("(t p) m -> t p m", p=P)
        wvr = wvT[:].rearrange("(t p) m -> t p m", p=P)
        wpr = wpT[:].rearrange("(t p) m -> t p m", p=P)

        # Inputs split across BOTH HWDGE trigger queues (sync + scalar) for
        # parallel DMA rings; batched 3D descriptors. x on sync; pair-0
        # weights on scalar so qk(0)/v(0) unblock at ~x-landing time.
        nc.sync.dma_start(out=xT_sb, in_=xTr.rearrange("t p n -> p t n"))
        wqk_t = wqkr.rearrange("t p m -> p t m")
        wv_t = wvr.rearrange("t p m -> p t m")
        nc.scalar.dma_start(out=wqkT_sb[:, :, 0:256], in_=wqk_t[:, :, 0:256])
        nc.scalar.dma_start(out=wvT_sb[:, :, 0:P], in_=wv_t[:, :, 0:P])
        for t in range(1, NPAIR):
            eng = nc.sync if t % 2 else nc.scalar
            eng.dma_start(
                out=wqkT_sb[:, :, t * 256:(t + 1) * 256],
                in_=wqk_t[:, :, t * 256:(t + 1) * 256],
            )
            eng.dma_start(
                out=wvT_sb[:, :, t * P:(t + 1) * P],
                in_=wv_t[:, :, t * P:(t + 1) * P],
            )
        nc.scalar.dma_start(out=wpT_sb, in_=wpr.rearrange("t p m -> p t m"))

        # ---- PE work generators (filler units of ~0.5-1.3us of matmuls) ----
        def gen_qk(t):
            """qk pair-tile t -> qkT_sb[:, 2t] (q) and [:, 2t+1] (k)."""
            for which in range(2):
                for lo, hi in _chunks(N, 512):
                    ps = s1ps.tile([P, 512], FP, tag="s1")
                    for k in range(KT):
                        nc.tensor.matmul(
                            ps,
                            wqkT_sb[:, k, t * 256 + which * P:
                                    t * 256 + (which + 1) * P],
                            xT_sb[:, k, lo:hi],
                            start=(k == 0),
                            stop=(k == KT - 1),
                        )
                    nc.vector.tensor_copy(
                        out=qkT_sb[:, 2 * t + which, lo:hi], in_=ps)
                    yield

        def gen_v(t):
            """v pair-slice t -> v_sb[:, :, 2t:2t+2, 0:D]."""
            for half in range(2):
                ps = s1ps.tile([P, 512], FP, tag="s1")
                for jj in range(4):
                    j = half * 4 + jj
                    for k in range(KT):
                        nc.tensor.matmul(
                            ps[:, jj * P:(jj + 1) * P],
                            xT_sb[:, k, j * P:(j + 1) * P],
                            wvT_sb[:, k, t * P:(t + 1) * P],
                            start=(k == 0),
                            stop=(k == KT - 1),
                        )
                    yield
                nc.vector.tensor_copy(
                    out=v_sb[:, half * 4:(half + 1) * 4, 2 * t:2 * t + 2, 0:D],
                    in_=ps.rearrange("p (j g d) -> p j g d", g=2, d=D),
                )

        def head_attn(h, filler, pending_rchain):
            """Attention for head h; pulls PE filler between steps.
            Issues its first two STs BEFORE running the previous head's
            r-chain (so ScalarE never starves at head boundaries), and
            returns its own r-chain as a closure for the next head."""
            t, hp = divmod(h, 2)
            hp *= D
            qT = qkT_sb[hp:hp + D, 2 * t]
            kT = qkT_sb[hp:hp + D, 2 * t + 1]
            # two 1-bank OT chunks (i cols 0:512 / 512:1024); a 3-slot pool
            # lets the next head's PV start while this head's r-chain runs
            ota = otps.tile([D + 1, 512], FP, tag="ot")
            otb = otps.tile([D + 1, 512], FP, tag="ot")
            ots = (ota, otb)

            def issue_st(j):
                st = stps.tile([P, N], FP, tag="st")
                ex = expp.tile([P, N], MMDT, tag="exp")
                for lo, hi in _chunks(N, 512):
                    nc.tensor.matmul(
                        st[:, lo:hi],
                        kT[:, j * P:(j + 1) * P],
                        qT[:, lo:hi],
                        start=True,
                        stop=True,
                    )
                nc.scalar.activation(
                    out=ex, in_=st, func=Exp, scale=float(SCALE),
                )
                return ex

            def issue_pv(j, ex):
                for c, (lo, hi) in enumerate(_chunks(N, 512)):
                    nc.tensor.matmul(
                        ots[c],
                        v_sb[:, j, h, :],
                        ex[:, lo:hi],
                        start=(j == 0),
                        stop=(j == NT - 1),
                    )

            def pull():
                try:
                    next(filler)
                except StopIteration:
                    pass

            exps = [issue_st(0), issue_st(1)]
            if pending_rchain is not None:
                pending_rchain()
            for j in range(NT):
                issue_pv(j, exps[j])
                pull()
                if j + 2 < NT:
                    exps.append(issue_st(j + 2))

            def rchain():
                # l rows (f32r) -> rank-1 ones⊗l broadcast into a PSUM slot
                # shared with the ST pool -> approx reciprocal (fp32) ->
                # normalize fused into the OT evacuation (bf16 out).
                la = rp.tile([1, 512], mybir.dt.float32r, tag="lrowa")
                lb_r = rp.tile([1, 512], mybir.dt.float32r, tag="lrowb")
                nc.vector.tensor_copy(out=la, in_=ota[D:D + 1, :])
                nc.vector.tensor_copy(out=lb_r, in_=otb[D:D + 1, :])
                pull()
                lbc = stps.tile([P, N], FP, tag="st")
                nc.tensor.matmul(lbc[:, 0:512], ones_f32r, la,
                                 start=True, stop=True)
                nc.tensor.matmul(lbc[:, 512:N], ones_f32r, lb_r,
                                 start=True, stop=True)
                rb_sb = rp.tile([P, N], FP, tag="rb")
                nc.vector.reciprocal_approx_fast(out=rb_sb, in_=lbc)
                nc.vector.tensor_tensor(
                    out=oT_sb[hp:hp + D, t, 0:512], in0=ota[0:D],
                    in1=rb_sb[0:D, 0:512], op=Mult,
                )
                nc.vector.tensor_tensor(
                    out=oT_sb[hp:hp + D, t, 512:N], in0=otb[0:D],
                    in1=rb_sb[0:D, 512:N], op=Mult,
                )

            return rchain

        def gen_proj_partial():
            """Output-projection contributions of k-tiles 0..4 (pairs 0-4),
            SBUF-accumulated into y_acc; runs as PE filler during pair 5 so
            only the thin k=5 pass remains after the last head."""
            for i in range(NT):
                for lo, hi in _chunks(DIM, 512):
                    ps = s1ps.tile([P, 512], FP, tag="s1")
                    for k in range(KT - 1):
                        nc.tensor.matmul(
                            ps[:, 0:hi - lo],
                            oT_sb[:, k, i * P:(i + 1) * P],
                            wpT_sb[:, k, lo:hi],
                            start=(k == 0),
                            stop=(k == KT - 2),
                        )
                    nc.vector.tensor_copy(
                        out=y_acc[:, i, lo:hi], in_=ps[:, 0:hi - lo])
                    yield

        # ---- interleaved pair loop ----
        def filler_for_pair(t):
            # spread filler units over the 18+ pulls of two heads by
            # inserting pacing skips
            if t + 1 < NPAIR:
                def units():
                    yield from gen_qk(t + 1)
                    yield from gen_v(t + 1)
                for i, u in enumerate(units()):
                    yield u
                    if i % 4 == 3:
                        yield None  # pacing skip
            else:
                for u in gen_proj_partial():
                    yield u

        for _ in gen_qk(0):
            pass
        for _ in gen_v(0):
            pass
        pending = None
        for t in range(NPAIR):
            f = filler_for_pair(t)
            pending = head_attn(2 * t, f, pending)
            pending = head_attn(2 * t + 1, f, pending)
            for _ in f:
                pass
        pending()

      # -------- stage 3: last projection k-tile (5) + combine --------
      with (
            tc.tile_pool(name="s3y", bufs=4) as s3y,
            tc.tile_pool(name="s3ps", bufs=2, space="PSUM") as s3ps,
      ):
            yr = y[:].rearrange("(i p) e -> i p e", p=P)
            for i in range(NT):
                ps = s3ps.tile([P, DIM], FP, tag="y")
                for lo, hi in _chunks(DIM, 512):
                    nc.tensor.matmul(
                        ps[:, lo:hi],
                        oT_sb[:, KT - 1, i * P:(i + 1) * P],
                        wpT_sb[:, KT - 1, lo:hi],
                        start=True,
                        stop=True,
                    )
                y_sb = s3y.tile([P, DIM], FP, tag="ysb")
                nc.vector.tensor_tensor(
                    out=y_sb, in0=ps, in1=y_acc[:, i], op=Add,
                )
                nc.vector.tensor_tensor(
                    out=y_sb, in0=y_sb, in1=bias_sb, op=Add,
                )
                nc.sync.dma_start(out=yr[i], in_=y_sb)


def prep_inputs(x, w_qkv, w_proj, b_proj):
    x = np.asarray(x, dtype=np.float32)
    w_qkv = np.asarray(w_qkv, dtype=np.float32)
    w_proj = np.asarray(w_proj, dtype=np.float32)
    b_proj = np.asarray(b_proj, dtype=np.float32)

    w_r = w_qkv.reshape(H, D, 3, DIM)  # rows ordered (h, d, qkv)
    wq = w_r[:, :, 0, :].reshape(DIM, DIM)  # rows (h, d)
    wk = w_r[:, :, 1, :].reshape(DIM, DIM)
    wv = w_r[:, :, 2, :].reshape(DIM, DIM)
    # pair-blocked qk: columns [q_t (128) | k_t (128)] for t = 0..5
    wqk_pairs = np.empty((2 * DIM, DIM), dtype=np.float32)
    for t in range(NPAIR):
        wqk_pairs[t * 256:t * 256 + P] = wq[t * P:(t + 1) * P]
        wqk_pairs[t * 256 + P:(t + 1) * 256] = wk[t * P:(t + 1) * P]
    wqkT = np.ascontiguousarray(wqk_pairs.T).astype(NP_MMDT)    # [768, 1536]
    wvT = np.ascontiguousarray(wv.T).astype(NP_MMDT)            # [768, 768]
    wpT = np.ascontiguousarray(w_proj.T).astype(NP_MMDT)        # [768, 768]
    xT = np.ascontiguousarray(x.transpose(0, 2, 1)).astype(NP_MMDT)
    bias = np.ascontiguousarray(b_proj.reshape(1, DIM))
    return xT, wqkT, wvT, wpT, bias


_NC = None
last_results = None


def get_nc():
    global _NC
    if _NC is None:
        _NC = build_nc()
    return _NC


def kernel(x, w_qkv, w_proj, b_proj):
    global last_results
    from concourse.bass_utils import run_bass_kernel_spmd

    nc = get_nc()
    xT, wqkT, wvT, wpT, bias = prep_inputs(x, w_qkv, w_proj, b_proj)
    in_maps = [
        {"xT": xT[c], "wqkT": wqkT, "wvT": wvT, "wpT": wpT, "bias": bias}
        for c in range(B)
    ]
    res = run_bass_kernel_spmd(nc, in_maps, core_ids=list(range(B)))
    last_results = res
    return np.stack([res.results[c]["y"] for c in range(B)], axis=0)
